# revision 34
# baseline (speedup 1.0000x reference)
"""Trainium2 Bass kernel for nn_BertAttentionDistance (B=4, S=2048, H=1024, NH=1, DT=32).

Sharding: 8 cores = (batch b = c//2) x (query-half qh = c%2, 1024 q-rows each).

Math notes (exact identities w.r.t. the reference):
  * Projection folding (v3): with q = hs Wq^T + bq, k = hs Wk^T + bk,
      q @ k^T = hs (Wq^T Wk) hs^T            [A: device, M := Wq^T Wk host]
              + (hs Wq^T bk)[q] * 1[k]       [B: per-q const -> cancels in softmax]
              + (hs Wk^T bq)[k]              [C: per-k const -> folded into exp bias]
              + bq.bk                        [D: const -> cancels]
    so the K-projection disappears; the q2 = hs M projection replaces the
    Q-projection at equal cost.
  * V/out folding: softmax rows sum to 1, so
      ctx Wo^T + bo = (probs @ hs) (Wo Wv)^T + (Wo bv + bo)
    with W2 := Wo Wv host-precomputed; PV contracts probs against RAW hs
    and the V-projection disappears. (Wo bv + bo) folds into the residual.
  * take_along_axis(word_dot_distance, rel, 3) * (rel == 1)
      == (q . dist_emb[1]) * (rel == 1)      (gather collapses)
  * wdd1 = q . dist_emb[1] = hs @ (Wq.T @ dist_emb[1]) + bq . dist_emb[1]
      computed on HOST, shipped as D[k, q] = (rel[q,k]==1) * wdd1[q] fp8;
      device adds D into the QK PSUM with an identity matmul (ps += I^T@D).
  * softmax max-subtraction skipped: scores/32 is O(+-3), safe in fp32 exp.
  * denominator: DR row-sum matmuls interleaved into the scores loop;
      1/denom applied at the out-projection evacuation.
  * all matmuls fp8 DoubleRow, fp32 PSUM. M and W2 are shipped x64 in fp8;
      raw hs / hs^T are natural fp8 range.

Per-core matmul count: q2 64 + scores (80+8)x2 + PV 64x2 + outproj 32x2
  + rq 8 = 440 DR MMs (vs 690 in the projection-based version).

DMA: all big sources host-swizzled so both AP sides have >=2KB contiguous
runs; first-need pieces split across the sync/scalar HWDGE queues.
"""

import sys

for p in ("/opt/trn_rl_repo", "/opt/pypackages"):
    if p not in sys.path:
        sys.path.insert(0, p)

from contextlib import ExitStack

import ml_dtypes
import numpy as np

import concourse.bacc as bacc
import concourse.bass as bass
import concourse.tile as tile
from concourse import mybir
from concourse.bass_utils import run_bass_kernel_spmd

# Problem constants (hardcoded per the harness contract).
B, S, H = 4, 2048, 1024
N_CORES = 8
SQ = 1024          # query rows per core
HC = H // 128      # 8 chunks of 128 over hidden dim
KC = S // 128      # 16 chunks of 128 over key dim
QN = SQ // 512     # 2 chunks of 512 over this core's query rows
LN_EPS = 1e-12
INV_SQRT_DH = 1.0 / 32.0
CTX_SCALE = 2.0 ** -5       # PV evacuation scale (ctxT into fp8 range)
W_SCALE = 64.0              # M / W2 fp8 scale (entries ~N(0, 0.013^2))
DN_SCALE = CTX_SCALE * W_SCALE    # fold into denominator before reciprocal

F32 = mybir.dt.float32
BF16 = mybir.dt.bfloat16
F8 = mybir.dt.float8e4
DR = mybir.MatmulPerfMode.DoubleRow
AF = mybir.ActivationFunctionType
ALU = mybir.AluOpType

_CACHE = {}


def _build_program(ln_affine=True):
    nc = bacc.Bacc("TRN2", target_bir_lowering=False, debug=False)

    # --- swizzled inputs (see make_in_maps for host layouts) ---
    # scores stationary: raw hs^T in kc-blocked layout [p][sc][c][j],
    # fully contiguous per partition (512B-segment loads were too slow)
    hsT_d = nc.dram_tensor("hsT", [128, KC * HC * 128], F8,
                           kind="ExternalInput")
    # PV stationary: raw hs rows [p(s)][kc*H + h]
    hsk_d = nc.dram_tensor("hsk", [128, KC * H], F8, kind="ExternalInput")
    # q2-proj moving: own-rows hs^T [half][p][c*512]
    hsqT_d = nc.dram_tensor("hsqT", [2, 128, HC * 512], F8,
                            kind="ExternalInput")
    hsq = nc.dram_tensor("hsq", [SQ, H], BF16, kind="ExternalInput")
    # distance pieces: [qn][quarter][p][4*512 contiguous]
    dT_d = nc.dram_tensor("dT", [QN, 4, 128, 4 * 512], F8, kind="ExternalInput")
    # M = Wq^T Wk (mc-major blocks, x64 fp8); W2^T = (Wo Wv)^T halves
    mT = nc.dram_tensor("mT", [128, HC * HC * 128], F8, kind="ExternalInput")
    w2T = nc.dram_tensor("w2T", [2, 128, HC * 512], F8, kind="ExternalInput")
    # [I,0] / [0,I] stacked DR identities for the distance-insert matmul
    ida_d = nc.dram_tensor("ida", [128, 2, 128], F8, kind="ExternalInput")
    idb_d = nc.dram_tensor("idb", [128, 2, 128], F8, kind="ExternalInput")
    # attention mask + (hs Wk^T bq)/32 per-k bias
    am_d = nc.dram_tensor("am", [128, KC], F32, kind="ExternalInput")
    lng_d = nc.dram_tensor("lng", [H], F32, kind="ExternalInput")
    lnb_d = nc.dram_tensor("lnb", [H], F32, kind="ExternalInput")
    out_d = nc.dram_tensor("out", [SQ, H], BF16, kind="ExternalOutput")

    def bcast_rows(src_1d_ap, p=128):
        """AP that reads a 1-D DRAM row broadcast across p partitions."""
        return bass.AP(
            tensor=src_1d_ap.tensor,
            offset=src_1d_ap.offset,
            ap=[[0, p], *src_1d_ap.ap],
        )

    with tile.TileContext(nc) as tc, ExitStack() as ctx:
        consts = ctx.enter_context(tc.tile_pool(name="consts", bufs=1))
        big = ctx.enter_context(tc.tile_pool(name="big", bufs=1))

        # ---- constants ----
        ones2 = consts.tile([128, 2, 128], F8)
        nc.vector.memset(ones2, 1.0)
        onef = consts.tile([1, 1], BF16)
        nc.vector.memset(onef, 1.0)
        eps_t = consts.tile([128, 1], F32)
        nc.vector.memset(eps_t, LN_EPS)
        ida_t = consts.tile([128, 2, 128], F8)
        idb_t = consts.tile([128, 2, 128], F8)
        am_t = consts.tile([128, KC], F32)
        nc.gpsimd.dma_start(ida_t, ida_d[:])
        nc.gpsimd.dma_start(idb_t, idb_d[:])
        nc.gpsimd.dma_start(am_t, am_d[:])
        if ln_affine:
            g_b = consts.tile([128, H], F32)
            nc.gpsimd.dma_start(g_b, bcast_rows(lng_d[:]))
            b_b = consts.tile([128, H], F32)
            nc.gpsimd.dma_start(b_b, bcast_rows(lnb_d[:]))

        # ---- persistent big tensors ----
        hsT_sb = big.tile([128, KC, HC, 128], F8)  # [h-part][kc][c][j]
        hsk_sb = big.tile([128, KC, H], F8)    # [s-part, kc, h] raw hs
        ctxT = big.tile([128, HC, SQ], F8)     # [h-part, c, q] (scaled 2^-5)
        rq_sb = big.tile([128, QN, 4], F32)    # 1/(denom*DN_SCALE)

        with (
            tc.tile_pool(name="qpool", bufs=1) as qpool,
            tc.tile_pool(name="dpool", bufs=2) as dpool,
        ):
            q2T = qpool.tile([128, HC, SQ], F8)    # [h-part, c, q]

            def dma_d(qn):
                d_all = dpool.tile([128, KC, 512], F8, tag="d")
                d_flat = d_all[:].rearrange("p k j -> p (k j)")
                for quarter in range(4):
                    nc.sync.dma_start(
                        d_flat[:, quarter * 2048:(quarter + 1) * 2048],
                        dT_d[qn, quarter],
                    )
                return d_all

            # ================= phase 1: q2 projection =================
            with (
                tc.tile_pool(name="wpool", bufs=1) as wpool,
                tc.tile_pool(name="psum_p", bufs=4, space="PSUM") as psum_p,
            ):
                # first-need: m mc-piece0 (scalar, 256KB) + hsqT h0 (sync)
                # in parallel; bulk flows behind them on the same queues
                m_sb = wpool.tile([128, HC, HC, 128], F8)   # [p][mc][c][j]
                m_flat = m_sb[:].rearrange("p m c j -> p (m c j)")
                hq_sb = wpool.tile([128, 2, HC, 512], F8)
                hq_flat = hq_sb[:].rearrange("p h c j -> p (h c j)")
                nc.sync.dma_start(hq_flat[:, 0:4096], hsqT_d[0])
                for i in range(4):
                    nc.scalar.dma_start(m_flat[:, i * 2048:(i + 1) * 2048],
                                        mT[:, i * 2048:(i + 1) * 2048])
                nc.sync.dma_start(hq_flat[:, 4096:8192], hsqT_d[1])
                # qn=0 distance tiles (needed ~26us): sync queue
                d_all0 = dma_d(0)
                # scores stationary (needed ~26us+): scalar queue
                hsT_flat = hsT_sb[:].rearrange("p s c j -> p (s c j)")
                for i in range(4):
                    nc.scalar.dma_start(
                        hsT_flat[:, i * 4096:(i + 1) * 4096],
                        hsT_d[:, i * 4096:(i + 1) * 4096],
                    )
                # PV stationary (needed ~42us): gpsimd queue, two pieces
                hsk_flat = hsk_sb[:].rearrange("p k h -> p (k h)")
                for k0 in range(0, KC, 8):
                    nc.gpsimd.dma_start(
                        hsk_flat[:, k0 * H:(k0 + 8) * H],
                        hsk_d[:, k0 * H:(k0 + 8) * H],
                    )

                # q2[n, m] = hs_own @ M : dst q2T[:, mc, n0*512...]
                for n0 in range(2):
                    for mc in range(HC):
                        ps = psum_p.tile([128, 512], F32, tag="pp")
                        for dc in range(0, HC, 2):
                            nc.tensor.matmul(
                                ps,
                                m_sb[:, mc, dc:dc + 2, :],
                                hq_sb[:, n0, dc:dc + 2, :],
                                start=(dc == 0),
                                stop=(dc == HC - 2),
                                perf_mode=DR,
                            )
                        nc.scalar.mul(q2T[:, mc, n0 * 512:(n0 + 1) * 512],
                                      ps, 1.0 / W_SCALE)

            # ====== phase 2+3 interleaved per query-chunk of 512 ======
            with (
                tc.tile_pool(name="expp", bufs=2) as expp,
                tc.tile_pool(name="wo_pool", bufs=1) as wo_pool,
                tc.tile_pool(name="epi", bufs=1) as epi,
                tc.tile_pool(name="sqp", bufs=2) as sqp,
                tc.tile_pool(name="hqp", bufs=2) as hqp,
                tc.tile_pool(name="denp", bufs=2) as denp,
                tc.tile_pool(name="stat", bufs=1) as stat,
                tc.tile_pool(name="psum_s", bufs=4, space="PSUM") as psum_s,
                tc.tile_pool(name="psum_v", bufs=2, space="PSUM") as psum_v,
                tc.tile_pool(name="psum_sm", bufs=1, space="PSUM") as psum_sm,
            ):
                wo_sb = wo_pool.tile([128, 2, HC, 512], F8)

                def dma_wo():
                    wo_flat = wo_sb[:].rearrange("p h c j -> p (h c j)")
                    nc.sync.dma_start(wo_flat[:, 0:4096], w2T[0])
                    nc.sync.dma_start(wo_flat[:, 4096:8192], w2T[1])

                def scores_phase(qn, d_all):
                    # hs^T @ q2 matmuls + D insert + exp; denominator
                    # row-sum matmuls (DR pairs) interleaved.
                    q_sl = slice(qn * 512, (qn + 1) * 512)
                    expT = expp.tile([128, KC, 512], F8, tag="expT")
                    hsq_t = hqp.tile([128, 4, H], BF16, tag="hsq")
                    for hhalf in range(2):
                        r0 = qn * 512 + hhalf * 256
                        nc.scalar.dma_start(
                            hsq_t[:, hhalf * 2:hhalf * 2 + 2, :],
                            hsq[:].rearrange("(c p) h -> p c h", p=128)[
                                :, r0 // 128:r0 // 128 + 2, :
                            ],
                        )
                    dn = psum_sm.tile([128, 512], F32, tag="dn")
                    for kc in range(KC):
                        ps = psum_s.tile([128, 512], F32, tag="ps")
                        for dc in range(0, HC, 2):
                            nc.tensor.matmul(
                                ps,
                                hsT_sb[:, kc, dc:dc + 2, :],
                                q2T[:, dc:dc + 2, q_sl],
                                start=(dc == 0),
                                stop=False,
                                perf_mode=DR,
                            )

                        # += distance term via a DR identity matmul
                        kc0 = kc - (kc % 2)
                        nc.tensor.matmul(
                            ps,
                            ida_t if kc % 2 == 0 else idb_t,
                            d_all[:, kc0:kc0 + 2, :],
                            start=False, stop=True,
                            perf_mode=DR,
                        )
                        # exp((qk + dist)/32 + am[k] + (hs Wk^T bq)[k]/32)
                        nc.scalar.activation(
                            expT[:, kc, :],
                            ps,
                            AF.Exp,
                            bias=am_t[:, kc:kc + 1],
                            scale=INV_SQRT_DH,
                        )
                        if kc % 2 == 1:
                            nc.tensor.matmul(
                                dn,
                                ones2,
                                expT[:, kc - 1:kc + 1, :],
                                start=(kc == 1),
                                stop=(kc == KC - 1),
                                perf_mode=DR,
                            )
                    # bf16 so the rq-transpose matmuls run single-pass
                    # (fp32 lhsT forces the LOW/HIGH double-pass on PE)
                    dn_sb = denp.tile([1, 512], BF16, tag="dr")
                    nc.scalar.mul(dn_sb, dn[0:1, :], DN_SCALE)
                    return expT, (dn_sb, dn), hsq_t

                def pv_phase(qn, expT, dn_sb, dn):
                    # PV: ctxT[h, q] = (hs^T-by-k @ expT) * 2^-5  (fp8)
                    q_sl = slice(qn * 512, (qn + 1) * 512)
                    for dc in range(HC):
                        pv = psum_v.tile([128, 512], F32, tag="pv")
                        for kc in range(0, KC, 2):
                            nc.tensor.matmul(
                                pv,
                                hsk_sb[:, kc:kc + 2, dc * 128:(dc + 1) * 128],
                                expT[:, kc:kc + 2, :],
                                start=(kc == 0),
                                stop=(kc == KC - 2),
                                perf_mode=DR,
                            )
                        nc.scalar.mul(ctxT[:, dc, q_sl], pv, CTX_SCALE)
                    # rq[q] = 1/(denom * DN_SCALE) transposed to q-partitions
                    rqp = dn[:, 0:4]
                    for j in range(4):
                        nc.tensor.matmul(
                            rqp[:, j:j + 1],
                            dn_sb[0:1, j * 128:(j + 1) * 128],
                            onef,
                            start=True,
                            stop=True,
                        )
                    nc.vector.reciprocal(rq_sb[:, qn, :], rqp)

                def outproj_mms(qn, si, hn):
                    sc = qn * 4 + si
                    ao = psum_v.tile([128, 512], F32, tag="pv")
                    for dc in range(0, HC, 2):
                        nc.tensor.matmul(
                            ao,
                            ctxT[:, dc:dc + 2, sc * 128:(sc + 1) * 128],
                            wo_sb[:, hn, dc:dc + 2, :],
                            start=(dc == 0),
                            stop=(dc == HC - 2),
                            perf_mode=DR,
                        )
                    return ao

                def epilogue(qn, hsq_t):
                    # out-proj (fp8 DR) + 1/denom + residual + LayerNorm.
                    xs, sts, mvs, rrs = [], [], [], []
                    for si, sc in enumerate(range(qn * 4, qn * 4 + 4)):
                        x_t = epi.tile([128, H], F32, tag=f"x{si}")
                        xs.append(x_t)
                        for hn in range(2):
                            ao = outproj_mms(qn, si, hn)
                            h_sl = slice(hn * 512, (hn + 1) * 512)
                            nc.vector.scalar_tensor_tensor(
                                x_t[:, h_sl], ao,
                                rq_sb[:, qn, si:si + 1],
                                hsq_t[:, si, h_sl],
                                ALU.mult, ALU.add,
                            )
                    for si in range(4):
                        st = stat.tile([128, 2, 6], F32, tag=f"st{si}")
                        nc.vector.bn_stats(st[:, 0, :], xs[si][:, 0:512])
                        nc.vector.bn_stats(st[:, 1, :], xs[si][:, 512:1024])
                        sts.append(st)
                    for si in range(4):
                        mv = stat.tile([128, 2], F32, tag=f"mv{si}")
                        nc.vector.bn_aggr(mv, sts[si])
                        mvs.append(mv)
                        sd = stat.tile([128, 1], F32, tag=f"sd{si}")
                        nc.scalar.activation(
                            sd, mv[:, 1:2], AF.Sqrt, bias=eps_t,
                        )
                        rr = stat.tile([128, 1], F32, tag=f"rr{si}")
                        nc.vector.reciprocal(rr, sd)
                        rrs.append(rr)
                    for si, sc in enumerate(range(qn * 4, qn * 4 + 4)):
                        y_t = epi.tile([128, H], BF16, tag=f"y{si}")
                        nc.vector.tensor_scalar(
                            y_t, xs[si], mvs[si][:, 0:1], rrs[si],
                            ALU.subtract, ALU.mult,
                        )
                        if ln_affine:
                            nc.vector.tensor_mul(y_t, y_t, g_b)
                            nc.vector.tensor_add(y_t, y_t, b_b)
                        nc.sync.dma_start(out_d[sc * 128:(sc + 1) * 128, :], y_t)

                def epilogue_tail(qn, hsq_t):
                    # Final epilogue: per-si chains, DVE/ACT balanced
                    # (~2.9us each). sum(x) from the stt accum (DVE),
                    # sum(x^2) from one full-row Square accum (ACT);
                    # normalize split: half on DVE ts, half on ACT.
                    for si, sc in enumerate(range(qn * 4, qn * 4 + 4)):
                        x_t = epi.tile([128, H], F32, tag=f"x{si}")
                        s12 = stat.tile([128, 4], F32, tag=f"s{si}")
                        for hn in range(2):
                            ao = outproj_mms(qn, si, hn)
                            h_sl = slice(hn * 512, (hn + 1) * 512)
                            nc.vector.scalar_tensor_tensor(
                                x_t[:, h_sl], ao,
                                rq_sb[:, qn, si:si + 1],
                                hsq_t[:, si, h_sl],
                                ALU.mult, ALU.add,
                                accum_out=s12[:, hn:hn + 1],
                            )
                            # Square per half: h0's runs during h1's MMs,
                            # keeping only h1's on the critical chain
                            sq = sqp.tile([128, 512], F32, tag="sq")
                            nc.scalar.activation(
                                sq, x_t[:, h_sl], AF.Square,
                                accum_out=s12[:, 2 + hn:3 + hn],
                            )
                        s1 = stat.tile([128, 1], F32, tag=f"a{si}")
                        nc.vector.tensor_add(s1, s12[:, 0:1], s12[:, 1:2])
                        s2 = stat.tile([128, 1], F32, tag=f"c{si}")
                        nc.vector.tensor_add(s2, s12[:, 2:3], s12[:, 3:4])
                        u = stat.tile([128, 1], F32, tag=f"u{si}")
                        nc.vector.scalar_tensor_tensor(
                            u, s1, 1.0 / H, s1, ALU.mult, ALU.mult)
                        nc.vector.tensor_sub(u, s2, u)
                        sd = stat.tile([128, 1], F32, tag=f"d{si}")
                        nc.scalar.activation(sd, u, AF.Sqrt,
                                             bias=eps_t, scale=1.0 / H)
                        rr = stat.tile([128, 1], F32, tag=f"r{si}")
                        nc.vector.reciprocal(rr, sd)
                        mu = stat.tile([128, 1], F32, tag=f"m{si}")
                        nc.vector.tensor_scalar_mul(mu, s1, 1.0 / H)
                        nb = stat.tile([128, 1], F32, tag=f"n{si}")
                        nc.vector.scalar_tensor_tensor(
                            nb, s1, -1.0 / H, rr, ALU.mult, ALU.mult)
                        y_t = epi.tile([128, H], BF16, tag=f"y{si}")
                        nc.vector.tensor_scalar(
                            y_t[:, 0:512], x_t[:, 0:512], mu, rr,
                            ALU.subtract, ALU.mult,
                        )
                        nc.scalar.activation(
                            y_t[:, 512:1024], x_t[:, 512:1024], AF.Identity,
                            bias=nb, scale=rr,
                        )
                        nc.sync.dma_start(out_d[sc * 128:(sc + 1) * 128, :], y_t)

                exp0, dn0, hsq0 = scores_phase(0, d_all0)
                dma_wo()
                pv_phase(0, exp0, dn0[0], dn0[1])
                epilogue(0, hsq0)
                exp1, dn1, hsq1 = scores_phase(1, dma_d(1))
                pv_phase(1, exp1, dn1[0], dn1[1])
                if ln_affine:
                    epilogue(1, hsq1)
                else:
                    epilogue_tail(1, hsq1)

    nc.compile()
    return nc


def get_program(ln_affine=True):
    key = ("nc", ln_affine)
    if key not in _CACHE:
        _CACHE[key] = _build_program(ln_affine)
    return _CACHE[key]


def _whalf(wT_scaled_f8):
    """[H, H] (contraction-major) -> [2, 128, HC*512] halves, contiguous.

    out[half, p, c*512 + j] = wT[c*128 + p, half*512 + j]
    """
    return np.ascontiguousarray(
        wT_scaled_f8.reshape(HC, 128, 2, 512).transpose(2, 1, 0, 3)
    ).reshape(2, 128, HC * 512)


def make_in_maps(inputs):
    """Host-side sharding / layout prep (numpy only)."""
    f32 = np.float32
    hs = np.asarray(inputs["hidden_states"], dtype=f32)
    rel = np.asarray(inputs["word_word_relation"])
    am = np.asarray(inputs["attention_mask"], dtype=f32)  # [B,1,1,S]
    Wq = np.asarray(inputs["Wq"], dtype=f32)
    Wk = np.asarray(inputs["Wk"], dtype=f32)
    Wv = np.asarray(inputs["Wv"], dtype=f32)
    Wo = np.asarray(inputs["Wo"], dtype=f32)
    bq = np.asarray(inputs["bq"], dtype=f32)
    bv = np.asarray(inputs["bv"], dtype=f32)
    bo = np.asarray(inputs["bo"], dtype=f32)
    d1 = np.asarray(inputs["dist_emb"], dtype=f32)[1]

    f8 = ml_dtypes.float8_e4m3
    # M = Wq^T Wk in mc-major blocks [p][mc][c][j]; W2^T = (Wo Wv)^T halves
    m8 = (Wq.T @ Wk * W_SCALE).astype(f8)
    m_h = np.ascontiguousarray(
        m8.reshape(HC, 128, HC, 128).transpose(1, 2, 0, 3)
    ).reshape(128, HC * HC * 128)
    w2_h = _whalf(((Wo @ Wv).T * W_SCALE).astype(f8))
    bo_eff = Wo @ bv + bo  # v/o biases fold into the residual
    # per-k score bias term C = hs Wk^T bq (enters exp bias /32)
    ck = hs @ (Wk.T @ bq)                              # [B, S]
    # wdd1[b, s] = q[b, s] . dist_emb[1], via associativity (host-cheap)
    wdd1 = hs @ (Wq.T @ d1) + float(bq @ d1)           # [B, S]
    eye = np.eye(128, dtype=np.float32)
    ida = np.zeros((128, 2, 128), dtype=np.float32)
    ida[:, 0, :] = eye
    idb = np.zeros((128, 2, 128), dtype=np.float32)
    idb[:, 1, :] = eye
    ida = ida.astype(f8)
    idb = idb.astype(f8)

    in_maps = []
    for c in range(N_CORES):
        b, qh = divmod(c, 2)
        qs = qh * SQ
        hsb8 = hs[b].astype(f8)                        # [S, H] fp8
        # hsT blocked [p][sc*1024 + c*128 + j] = hs[b, sc*128+j, c*128+p]
        hsT_img = np.ascontiguousarray(
            hsb8.reshape(KC, 128, HC, 128).transpose(3, 0, 2, 1)
        ).reshape(128, KC * HC * 128)
        # hsk [p][kc*H + h] = hs[b, kc*128+p, h]
        hsk = np.ascontiguousarray(
            hsb8.reshape(KC, 128, H).transpose(1, 0, 2)
        ).reshape(128, KC * H)
        # hsqT [half][p][c*512+j] = hs[b, qs + half*512 + j, c*128+p]
        hq8 = hsb8[qs:qs + SQ]                         # [SQ, H]
        hsqT = np.ascontiguousarray(
            hq8.T.reshape(HC, 128, 2, 512).transpose(2, 1, 0, 3)
        ).reshape(2, 128, HC * 512)
        mask_T = rel[b, qs:qs + SQ, :].T == 1          # [S(k), SQ(q)]
        dTm = (mask_T * wdd1[b, qs:qs + SQ][None, :]).astype(f8)
        dT_sw = np.ascontiguousarray(
            dTm.reshape(4, 4, 128, 2, 512)
            .transpose(3, 0, 2, 1, 4)
            .reshape(QN, 4, 128, 4 * 512)
        )
        amf = am[b, 0, 0] + ck[b] * INV_SQRT_DH        # [S] per-k exp bias
        in_maps.append({
            "hsT": hsT_img,
            "hsk": hsk,
            "hsqT": hsqT,
            "hsq": (hs[b, qs:qs + SQ, :] + bo_eff).astype(ml_dtypes.bfloat16),
            "dT": dT_sw,
            "mT": m_h, "w2T": w2_h,
            "ida": ida, "idb": idb,
            "am": np.ascontiguousarray(amf.reshape(KC, 128).T),
            "lng": np.asarray(inputs["ln_g"], dtype=f32),
            "lnb": np.asarray(inputs["ln_b"], dtype=f32),
        })
    return in_maps


def kernel(**inputs):
    ln_affine = not (
        np.all(np.asarray(inputs["ln_g"]) == 1.0)
        and np.all(np.asarray(inputs["ln_b"]) == 0.0)
    )
    nc = get_program(ln_affine)
    in_maps = make_in_maps(inputs)
    res = run_bass_kernel_spmd(nc, in_maps, core_ids=list(range(N_CORES)))
    out = np.empty((B, S, H), dtype=np.float32)
    for c in range(N_CORES):
        b, qh = divmod(c, 2)
        out[b, qh * SQ:(qh + 1) * SQ, :] = res.results[c]["out"].astype(
            np.float32
        )
    return out


# revision 44
# speedup vs baseline: 1.0454x; 1.0454x over previous
"""Trainium2 Bass kernel for nn_BertAttentionDistance (B=4, S=2048, H=1024, NH=1, DT=32).

Sharding: 8 cores = (batch b = c//2) x (query-half qh = c%2, 1024 q-rows each).

Math notes (exact identities w.r.t. the reference):
  * Projection folding (v3): with q = hs Wq^T + bq, k = hs Wk^T + bk,
      q @ k^T = hs (Wq^T Wk) hs^T            [A: device, M := Wq^T Wk host]
              + (hs Wq^T bk)[q] * 1[k]       [B: per-q const -> cancels in softmax]
              + (hs Wk^T bq)[k]              [C: per-k const -> folded into exp bias]
              + bq.bk                        [D: const -> cancels]
    so the K-projection disappears; the q2 = hs M projection replaces the
    Q-projection at equal cost.
  * V/out folding: softmax rows sum to 1, so
      ctx Wo^T + bo = (probs @ hs) (Wo Wv)^T + (Wo bv + bo)
    with W2 := Wo Wv host-precomputed; PV contracts probs against RAW hs
    and the V-projection disappears. (Wo bv + bo) folds into the residual.
  * take_along_axis(word_dot_distance, rel, 3) * (rel == 1)
      == (q . dist_emb[1]) * (rel == 1)      (gather collapses)
  * wdd1 = q . dist_emb[1] = hs @ (Wq.T @ dist_emb[1]) + bq . dist_emb[1]
      computed on HOST, shipped as D[k, q] = (rel[q,k]==1) * wdd1[q] fp8;
      device adds D into the QK PSUM with an identity matmul (ps += I^T@D).
  * softmax max-subtraction skipped: scores/32 is O(+-3), safe in fp32 exp.
  * denominator: DR row-sum matmuls interleaved into the scores loop;
      1/denom applied at the out-projection evacuation.
  * all matmuls fp8 DoubleRow, fp32 PSUM. M and W2 are shipped x64 in fp8;
      raw hs / hs^T are natural fp8 range.

Per-core matmul count: q2 64 + scores (80+8)x2 + PV 64x2 + outproj 32x2
  + rq 8 = 440 DR MMs (vs 690 in the projection-based version).

DMA: all big sources host-swizzled so both AP sides have >=2KB contiguous
runs; first-need pieces split across the sync/scalar HWDGE queues.
"""

import sys

for p in ("/opt/trn_rl_repo", "/opt/pypackages"):
    if p not in sys.path:
        sys.path.insert(0, p)

from contextlib import ExitStack

import ml_dtypes
import numpy as np

import concourse.bacc as bacc
import concourse.bass as bass
import concourse.tile as tile
from concourse import mybir
from concourse.bass_utils import run_bass_kernel_spmd

# Problem constants (hardcoded per the harness contract).
B, S, H = 4, 2048, 1024
N_CORES = 8
SQ = 1024          # query rows per core
HC = H // 128      # 8 chunks of 128 over hidden dim
KC = S // 128      # 16 chunks of 128 over key dim
QN = SQ // 512     # 2 chunks of 512 over this core's query rows
LN_EPS = 1e-12
INV_SQRT_DH = 1.0 / 32.0
CTX_SCALE = 2.0 ** -5       # PV evacuation scale (ctxT into fp8 range)
W_SCALE = 64.0              # M / W2 fp8 scale (entries ~N(0, 0.013^2))
DN_SCALE = CTX_SCALE * W_SCALE    # fold into denominator before reciprocal

F32 = mybir.dt.float32
BF16 = mybir.dt.bfloat16
F8 = mybir.dt.float8e4
DR = mybir.MatmulPerfMode.DoubleRow
AF = mybir.ActivationFunctionType
ALU = mybir.AluOpType

_CACHE = {}


def _build_program(ln_affine=True):
    nc = bacc.Bacc("TRN2", target_bir_lowering=False, debug=False)

    # --- swizzled inputs (see make_in_maps for host layouts) ---
    # scores stationary: raw hs^T SBUF image [p][c][s] (one full-tensor
    # DMA: 16KB/partition contiguous runs at full rate)
    hsT_d = nc.dram_tensor("hsT", [128, HC, S], F8, kind="ExternalInput")
    # PV stationary: raw hs rows [p(s)][kc*H + h]
    hsk_d = nc.dram_tensor("hsk", [128, KC * H], F8, kind="ExternalInput")
    # q2-proj moving: own-rows hs^T [half][p][c*512]
    hsqT_d = nc.dram_tensor("hsqT", [2, 128, HC * 512], F8,
                            kind="ExternalInput")
    hsq = nc.dram_tensor("hsq", [SQ, H], BF16, kind="ExternalInput")
    # distance pieces: [qn][quarter][p][4*512 contiguous]
    dT_d = nc.dram_tensor("dT", [QN, 4, 128, 4 * 512], F8, kind="ExternalInput")
    # M = Wq^T Wk and W2^T = (Wo Wv)^T, x64 fp8, [half][p][c*512]
    mT = nc.dram_tensor("mT", [2, 128, HC * 512], F8, kind="ExternalInput")
    w2T = nc.dram_tensor("w2T", [2, 128, HC * 512], F8, kind="ExternalInput")
    # [I,0] / [0,I] stacked DR identities for the distance-insert matmul
    ida_d = nc.dram_tensor("ida", [128, 2, 128], F8, kind="ExternalInput")
    idb_d = nc.dram_tensor("idb", [128, 2, 128], F8, kind="ExternalInput")
    # attention mask + (hs Wk^T bq)/32 per-k bias
    am_d = nc.dram_tensor("am", [128, KC], F32, kind="ExternalInput")
    lng_d = nc.dram_tensor("lng", [H], F32, kind="ExternalInput")
    lnb_d = nc.dram_tensor("lnb", [H], F32, kind="ExternalInput")
    out_d = nc.dram_tensor("out", [SQ, H], BF16, kind="ExternalOutput")

    def bcast_rows(src_1d_ap, p=128):
        """AP that reads a 1-D DRAM row broadcast across p partitions."""
        return bass.AP(
            tensor=src_1d_ap.tensor,
            offset=src_1d_ap.offset,
            ap=[[0, p], *src_1d_ap.ap],
        )

    with tile.TileContext(nc) as tc, ExitStack() as ctx:
        consts = ctx.enter_context(tc.tile_pool(name="consts", bufs=1))
        big = ctx.enter_context(tc.tile_pool(name="big", bufs=1))

        # ---- constants ----
        ones2 = consts.tile([128, 2, 128], F8)
        nc.vector.memset(ones2, 1.0)
        onef = consts.tile([1, 1], BF16)
        nc.vector.memset(onef, 1.0)
        eps_t = consts.tile([128, 1], F32)
        nc.vector.memset(eps_t, LN_EPS)
        ida_t = consts.tile([128, 2, 128], F8)
        idb_t = consts.tile([128, 2, 128], F8)
        am_t = consts.tile([128, KC], F32)
        nc.gpsimd.dma_start(ida_t, ida_d[:])
        nc.gpsimd.dma_start(idb_t, idb_d[:])
        nc.gpsimd.dma_start(am_t, am_d[:])
        if ln_affine:
            g_b = consts.tile([128, H], F32)
            nc.gpsimd.dma_start(g_b, bcast_rows(lng_d[:]))
            b_b = consts.tile([128, H], F32)
            nc.gpsimd.dma_start(b_b, bcast_rows(lnb_d[:]))

        # ---- persistent big tensors ----
        hsT_sb = big.tile([128, HC, S], F8)    # [h-part, c, k] raw hs^T
        hsk_sb = big.tile([128, KC, H], F8)    # [s-part, kc, h] raw hs
        ctxT = big.tile([128, HC, SQ], F8)     # [h-part, c, q] (scaled 2^-5)
        rq_sb = big.tile([128, QN, 4], F32)    # 1/(denom*DN_SCALE)

        with (
            tc.tile_pool(name="qpool", bufs=1) as qpool,
            tc.tile_pool(name="dpool", bufs=2) as dpool,
        ):
            q2T = qpool.tile([128, HC, SQ], F8)    # [h-part, c, q]

            def dma_d(qn):
                # gpsimd SWDGE queue: keeps the HWDGE queues free
                d_all = dpool.tile([128, KC, 512], F8, tag="d")
                d_flat = d_all[:].rearrange("p k j -> p (k j)")
                for quarter in range(4):
                    nc.gpsimd.dma_start(
                        d_flat[:, quarter * 2048:(quarter + 1) * 2048],
                        dT_d[qn, quarter],
                    )
                return d_all

            # ================= phase 1: q2 projection =================
            with (
                tc.tile_pool(name="wpool", bufs=1) as wpool,
                tc.tile_pool(name="psum_p", bufs=4, space="PSUM") as psum_p,
            ):
                # first-need: mT h0 (scalar) + hsqT h0 (sync) in parallel;
                # then the full hsT image (needed ~26us) at full rate
                m_sb = wpool.tile([128, 2, HC, 512], F8)
                m_flat = m_sb[:].rearrange("p h c j -> p (h c j)")
                hq_sb = wpool.tile([128, 2, HC, 512], F8)
                hq_flat = hq_sb[:].rearrange("p h c j -> p (h c j)")
                nc.sync.dma_start(hq_flat[:, 0:4096], hsqT_d[0])
                nc.scalar.dma_start(m_flat[:, 0:4096], mT[0])
                nc.sync.dma_start(
                    hsT_sb[:].rearrange("p c s -> p (c s)"),
                    hsT_d[:].rearrange("p c s -> p (c s)"),
                )
                nc.sync.dma_start(hq_flat[:, 4096:8192], hsqT_d[1])
                nc.scalar.dma_start(m_flat[:, 4096:8192], mT[1])
                # qn=0 distance tiles (needed ~26us): gpsimd queue
                d_all0 = dma_d(0)
                # PV stationary (needed ~42us): gpsimd queue, two pieces
                hsk_flat = hsk_sb[:].rearrange("p k h -> p (k h)")
                for k0 in range(0, KC, 8):
                    nc.gpsimd.dma_start(
                        hsk_flat[:, k0 * H:(k0 + 8) * H],
                        hsk_d[:, k0 * H:(k0 + 8) * H],
                    )

                # q2[n, m] = hs_own @ M : dst q2T[:, mc, n0*512...]
                for n0 in range(2):
                    for mc in range(HC):
                        ps = psum_p.tile([128, 512], F32, tag="pp")
                        for dc in range(0, HC, 2):
                            nc.tensor.matmul(
                                ps,
                                m_sb[:, mc // 4, dc:dc + 2,
                                     (mc % 4) * 128:(mc % 4 + 1) * 128],
                                hq_sb[:, n0, dc:dc + 2, :],
                                start=(dc == 0),
                                stop=(dc == HC - 2),
                                perf_mode=DR,
                            )
                        nc.scalar.mul(q2T[:, mc, n0 * 512:(n0 + 1) * 512],
                                      ps, 1.0 / W_SCALE)

            # ====== phase 2+3 interleaved per query-chunk of 512 ======
            with (
                tc.tile_pool(name="expp", bufs=2) as expp,
                tc.tile_pool(name="wo_pool", bufs=1) as wo_pool,
                tc.tile_pool(name="epi", bufs=1) as epi,
                tc.tile_pool(name="sqp", bufs=2) as sqp,
                tc.tile_pool(name="hqp", bufs=2) as hqp,
                tc.tile_pool(name="denp", bufs=2) as denp,
                tc.tile_pool(name="stat", bufs=1) as stat,
                tc.tile_pool(name="psum_s", bufs=4, space="PSUM") as psum_s,
                tc.tile_pool(name="psum_v", bufs=2, space="PSUM") as psum_v,
                tc.tile_pool(name="psum_sm", bufs=1, space="PSUM") as psum_sm,
            ):
                wo_sb = wo_pool.tile([128, 2, HC, 512], F8)

                def dma_wo():
                    wo_flat = wo_sb[:].rearrange("p h c j -> p (h c j)")
                    nc.sync.dma_start(wo_flat[:, 0:4096], w2T[0])
                    nc.sync.dma_start(wo_flat[:, 4096:8192], w2T[1])

                def scores_phase(qn, d_all):
                    # hs^T @ q2 matmuls + D insert + exp; denominator
                    # row-sum matmuls (DR pairs) interleaved.
                    q_sl = slice(qn * 512, (qn + 1) * 512)
                    expT = expp.tile([128, KC, 512], F8, tag="expT")
                    hsq_t = hqp.tile([128, 4, H], BF16, tag="hsq")
                    for hhalf in range(2):
                        r0 = qn * 512 + hhalf * 256
                        nc.scalar.dma_start(
                            hsq_t[:, hhalf * 2:hhalf * 2 + 2, :],
                            hsq[:].rearrange("(c p) h -> p c h", p=128)[
                                :, r0 // 128:r0 // 128 + 2, :
                            ],
                        )
                    dn = psum_sm.tile([128, 512], F32, tag="dn")
                    for kc in range(KC):
                        ps = psum_s.tile([128, 512], F32, tag="ps")
                        for dc in range(0, HC, 2):
                            nc.tensor.matmul(
                                ps,
                                hsT_sb[:, dc:dc + 2, kc * 128:(kc + 1) * 128],
                                q2T[:, dc:dc + 2, q_sl],
                                start=(dc == 0),
                                stop=False,
                                perf_mode=DR,
                            )
                        # += distance term via a DR identity matmul
                        kc0 = kc - (kc % 2)
                        nc.tensor.matmul(
                            ps,
                            ida_t if kc % 2 == 0 else idb_t,
                            d_all[:, kc0:kc0 + 2, :],
                            start=False, stop=True,
                            perf_mode=DR,
                        )
                        # exp((qk + dist)/32 + am[k] + (hs Wk^T bq)[k]/32)
                        nc.scalar.activation(
                            expT[:, kc, :],
                            ps,
                            AF.Exp,
                            bias=am_t[:, kc:kc + 1],
                            scale=INV_SQRT_DH,
                        )
                        if kc % 2 == 1:
                            nc.tensor.matmul(
                                dn,
                                ones2,
                                expT[:, kc - 1:kc + 1, :],
                                start=(kc == 1),
                                stop=(kc == KC - 1),
                                perf_mode=DR,
                            )
                    # bf16 so the rq-transpose matmuls run single-pass
                    # (fp32 lhsT forces the LOW/HIGH double-pass on PE)
                    dn_sb = denp.tile([1, 512], BF16, tag="dr")
                    nc.scalar.mul(dn_sb, dn[0:1, :], DN_SCALE)
                    return expT, (dn_sb, dn), hsq_t

                def pv_phase(qn, expT, dn_sb, dn):
                    # PV: ctxT[h, q] = (hs^T-by-k @ expT) * 2^-5  (fp8)
                    q_sl = slice(qn * 512, (qn + 1) * 512)
                    for dc in range(HC):
                        pv = psum_v.tile([128, 512], F32, tag="pv")
                        for kc in range(0, KC, 2):
                            nc.tensor.matmul(
                                pv,
                                hsk_sb[:, kc:kc + 2, dc * 128:(dc + 1) * 128],
                                expT[:, kc:kc + 2, :],
                                start=(kc == 0),
                                stop=(kc == KC - 2),
                                perf_mode=DR,
                            )
                        nc.scalar.mul(ctxT[:, dc, q_sl], pv, CTX_SCALE)
                    # rq[q] = 1/(denom * DN_SCALE) transposed to q-partitions
                    rqp = dn[:, 0:4]
                    for j in range(4):
                        nc.tensor.matmul(
                            rqp[:, j:j + 1],
                            dn_sb[0:1, j * 128:(j + 1) * 128],
                            onef,
                            start=True,
                            stop=True,
                        )
                    nc.vector.reciprocal(rq_sb[:, qn, :], rqp)

                def outproj_mms(qn, si, hn):
                    sc = qn * 4 + si
                    ao = psum_v.tile([128, 512], F32, tag="pv")
                    for dc in range(0, HC, 2):
                        nc.tensor.matmul(
                            ao,
                            ctxT[:, dc:dc + 2, sc * 128:(sc + 1) * 128],
                            wo_sb[:, hn, dc:dc + 2, :],
                            start=(dc == 0),
                            stop=(dc == HC - 2),
                            perf_mode=DR,
                        )
                    return ao

                def epilogue_x(qn, hsq_t):
                    # out-proj (fp8 DR) + 1/denom + residual -> x tiles.
                    # Emitted right after PV so the psum_v banks recycle.
                    xs = []
                    for si, sc in enumerate(range(qn * 4, qn * 4 + 4)):
                        x_t = epi.tile([128, H], F32, tag=f"x{si}")
                        xs.append(x_t)
                        for hn in range(2):
                            ao = outproj_mms(qn, si, hn)
                            h_sl = slice(hn * 512, (hn + 1) * 512)
                            nc.vector.scalar_tensor_tensor(
                                x_t[:, h_sl], ao,
                                rq_sb[:, qn, si:si + 1],
                                hsq_t[:, si, h_sl],
                                ALU.mult, ALU.add,
                            )
                    return xs

                def epilogue_ln(qn, xs):
                    # LayerNorm for epilogue(0), emitted AFTER scores(1) so
                    # its Sqrt ops (and the one-off ACT table-1 load) queue
                    # behind all of scores(1)'s exps instead of between them
                    sts, mvs, rrs = [], [], []
                    for si in range(4):
                        st = stat.tile([128, 2, 6], F32, tag=f"st{si}")
                        nc.vector.bn_stats(st[:, 0, :], xs[si][:, 0:512])
                        nc.vector.bn_stats(st[:, 1, :], xs[si][:, 512:1024])
                        sts.append(st)
                    for si in range(4):
                        mv = stat.tile([128, 2], F32, tag=f"mv{si}")
                        nc.vector.bn_aggr(mv, sts[si])
                        mvs.append(mv)
                        sd = stat.tile([128, 1], F32, tag=f"sd{si}")
                        nc.scalar.activation(
                            sd, mv[:, 1:2], AF.Sqrt, bias=eps_t,
                        )
                        rr = stat.tile([128, 1], F32, tag=f"rr{si}")
                        nc.vector.reciprocal(rr, sd)
                        rrs.append(rr)
                    for si, sc in enumerate(range(qn * 4, qn * 4 + 4)):
                        y_t = epi.tile([128, H], BF16, tag=f"y{si}")
                        nc.vector.tensor_scalar(
                            y_t, xs[si], mvs[si][:, 0:1], rrs[si],
                            ALU.subtract, ALU.mult,
                        )
                        if ln_affine:
                            nc.vector.tensor_mul(y_t, y_t, g_b)
                            nc.vector.tensor_add(y_t, y_t, b_b)
                        nc.sync.dma_start(out_d[sc * 128:(sc + 1) * 128, :], y_t)

                def epilogue_tail(qn, hsq_t):
                    # Final epilogue: per-si chains, DVE/ACT balanced
                    # (~2.9us each). sum(x) from the stt accum (DVE),
                    # sum(x^2) from one full-row Square accum (ACT);
                    # normalize split: half on DVE ts, half on ACT.
                    for si, sc in enumerate(range(qn * 4, qn * 4 + 4)):
                        x_t = epi.tile([128, H], F32, tag=f"x{si}")
                        s12 = stat.tile([128, 4], F32, tag=f"s{si}")
                        for hn in range(2):
                            ao = outproj_mms(qn, si, hn)
                            h_sl = slice(hn * 512, (hn + 1) * 512)
                            nc.vector.scalar_tensor_tensor(
                                x_t[:, h_sl], ao,
                                rq_sb[:, qn, si:si + 1],
                                hsq_t[:, si, h_sl],
                                ALU.mult, ALU.add,
                                accum_out=s12[:, hn:hn + 1],
                            )
                            # Square per half: h0's runs during h1's MMs,
                            # keeping only h1's on the critical chain
                            sq = sqp.tile([128, 512], F32, tag="sq")
                            nc.scalar.activation(
                                sq, x_t[:, h_sl], AF.Square,
                                accum_out=s12[:, 2 + hn:3 + hn],
                            )
                        s1 = stat.tile([128, 1], F32, tag=f"a{si}")
                        nc.vector.tensor_add(s1, s12[:, 0:1], s12[:, 1:2])
                        s2 = stat.tile([128, 1], F32, tag=f"c{si}")
                        nc.vector.tensor_add(s2, s12[:, 2:3], s12[:, 3:4])
                        u = stat.tile([128, 1], F32, tag=f"u{si}")
                        nc.vector.scalar_tensor_tensor(
                            u, s1, 1.0 / H, s1, ALU.mult, ALU.mult)
                        nc.vector.tensor_sub(u, s2, u)
                        sd = stat.tile([128, 1], F32, tag=f"d{si}")
                        nc.scalar.activation(sd, u, AF.Sqrt,
                                             bias=eps_t, scale=1.0 / H)
                        rr = stat.tile([128, 1], F32, tag=f"r{si}")
                        nc.vector.reciprocal(rr, sd)
                        mu = stat.tile([128, 1], F32, tag=f"m{si}")
                        nc.vector.tensor_scalar_mul(mu, s1, 1.0 / H)
                        nb = stat.tile([128, 1], F32, tag=f"n{si}")
                        nc.vector.scalar_tensor_tensor(
                            nb, s1, -1.0 / H, rr, ALU.mult, ALU.mult)
                        y_t = epi.tile([128, H], BF16, tag=f"y{si}")
                        nc.vector.tensor_scalar(
                            y_t[:, 0:512], x_t[:, 0:512], mu, rr,
                            ALU.subtract, ALU.mult,
                        )
                        nc.scalar.activation(
                            y_t[:, 512:1024], x_t[:, 512:1024], AF.Identity,
                            bias=nb, scale=rr,
                        )
                        nc.sync.dma_start(out_d[sc * 128:(sc + 1) * 128, :], y_t)

                exp0, dn0, hsq0 = scores_phase(0, d_all0)
                dma_wo()
                pv_phase(0, exp0, dn0[0], dn0[1])
                xs0 = epilogue_x(0, hsq0)
                exp1, dn1, hsq1 = scores_phase(1, dma_d(1))
                epilogue_ln(0, xs0)
                pv_phase(1, exp1, dn1[0], dn1[1])
                if ln_affine:
                    xs1 = epilogue_x(1, hsq1)
                    epilogue_ln(1, xs1)
                else:
                    epilogue_tail(1, hsq1)

    nc.compile()
    return nc


def get_program(ln_affine=True):
    key = ("nc", ln_affine)
    if key not in _CACHE:
        _CACHE[key] = _build_program(ln_affine)
    return _CACHE[key]


def _whalf(wT_scaled_f8):
    """[H, H] (contraction-major) -> [2, 128, HC*512] halves, contiguous.

    out[half, p, c*512 + j] = wT[c*128 + p, half*512 + j]
    """
    return np.ascontiguousarray(
        wT_scaled_f8.reshape(HC, 128, 2, 512).transpose(2, 1, 0, 3)
    ).reshape(2, 128, HC * 512)


def make_in_maps(inputs):
    """Host-side sharding / layout prep (numpy only)."""
    f32 = np.float32
    hs = np.asarray(inputs["hidden_states"], dtype=f32)
    rel = np.asarray(inputs["word_word_relation"])
    am = np.asarray(inputs["attention_mask"], dtype=f32)  # [B,1,1,S]
    Wq = np.asarray(inputs["Wq"], dtype=f32)
    Wk = np.asarray(inputs["Wk"], dtype=f32)
    Wv = np.asarray(inputs["Wv"], dtype=f32)
    Wo = np.asarray(inputs["Wo"], dtype=f32)
    bq = np.asarray(inputs["bq"], dtype=f32)
    bv = np.asarray(inputs["bv"], dtype=f32)
    bo = np.asarray(inputs["bo"], dtype=f32)
    d1 = np.asarray(inputs["dist_emb"], dtype=f32)[1]

    f8 = ml_dtypes.float8_e4m3
    # M = Wq^T Wk (scores core), W2^T = (Wo Wv)^T (fused PV/out weights)
    m_h = _whalf((Wq.T @ Wk * W_SCALE).astype(f8))
    w2_h = _whalf(((Wo @ Wv).T * W_SCALE).astype(f8))
    bo_eff = Wo @ bv + bo  # v/o biases fold into the residual
    # per-k score bias term C = hs Wk^T bq (enters exp bias /32)
    ck = hs @ (Wk.T @ bq)                              # [B, S]
    # wdd1[b, s] = q[b, s] . dist_emb[1], via associativity (host-cheap)
    wdd1 = hs @ (Wq.T @ d1) + float(bq @ d1)           # [B, S]
    eye = np.eye(128, dtype=np.float32)
    ida = np.zeros((128, 2, 128), dtype=np.float32)
    ida[:, 0, :] = eye
    idb = np.zeros((128, 2, 128), dtype=np.float32)
    idb[:, 1, :] = eye
    ida = ida.astype(f8)
    idb = idb.astype(f8)

    in_maps = []
    for c in range(N_CORES):
        b, qh = divmod(c, 2)
        qs = qh * SQ
        hsb8 = hs[b].astype(f8)                        # [S, H] fp8
        # hsT SBUF image [p][c][s]: hsT[c*128+p, s] = hs[b, s, c*128+p]
        hsT_img = np.ascontiguousarray(
            hsb8.T.reshape(HC, 128, S).transpose(1, 0, 2)
        )
        # hsk [p][kc*H + h] = hs[b, kc*128+p, h]
        hsk = np.ascontiguousarray(
            hsb8.reshape(KC, 128, H).transpose(1, 0, 2)
        ).reshape(128, KC * H)
        # hsqT [half][p][c*512+j] = hs[b, qs + half*512 + j, c*128+p]
        hq8 = hsb8[qs:qs + SQ]                         # [SQ, H]
        hsqT = np.ascontiguousarray(
            hq8.T.reshape(HC, 128, 2, 512).transpose(2, 1, 0, 3)
        ).reshape(2, 128, HC * 512)
        mask_T = rel[b, qs:qs + SQ, :].T == 1          # [S(k), SQ(q)]
        dTm = (mask_T * wdd1[b, qs:qs + SQ][None, :]).astype(f8)
        dT_sw = np.ascontiguousarray(
            dTm.reshape(4, 4, 128, 2, 512)
            .transpose(3, 0, 2, 1, 4)
            .reshape(QN, 4, 128, 4 * 512)
        )
        amf = am[b, 0, 0] + ck[b] * INV_SQRT_DH        # [S] per-k exp bias
        in_maps.append({
            "hsT": hsT_img,
            "hsk": hsk,
            "hsqT": hsqT,
            "hsq": (hs[b, qs:qs + SQ, :] + bo_eff).astype(ml_dtypes.bfloat16),
            "dT": dT_sw,
            "mT": m_h, "w2T": w2_h,
            "ida": ida, "idb": idb,
            "am": np.ascontiguousarray(amf.reshape(KC, 128).T),
            "lng": np.asarray(inputs["ln_g"], dtype=f32),
            "lnb": np.asarray(inputs["ln_b"], dtype=f32),
        })
    return in_maps


def kernel(**inputs):
    ln_affine = not (
        np.all(np.asarray(inputs["ln_g"]) == 1.0)
        and np.all(np.asarray(inputs["ln_b"]) == 0.0)
    )
    nc = get_program(ln_affine)
    in_maps = make_in_maps(inputs)
    res = run_bass_kernel_spmd(nc, in_maps, core_ids=list(range(N_CORES)))
    out = np.empty((B, S, H), dtype=np.float32)
    for c in range(N_CORES):
        b, qh = divmod(c, 2)
        out[b, qh * SQ:(qh + 1) * SQ, :] = res.results[c]["out"].astype(
            np.float32
        )
    return out
